# revision 1
# baseline (speedup 1.0000x reference)
"""Trainium2 Bass kernel for nn_AttentionBlock (GroupNorm + MHA + proj + residual).

Problem: x[8, 512, 32, 32] fp32; GroupNorm(32 groups) -> qkv (1x1 conv) ->
8-head attention over 1024 spatial positions -> proj -> residual.

Sharding: data-parallel over batch. 8 batch elements -> 8 NeuronCores,
one SPMD NEFF. No collectives.

v2 design (per-core, x as [c=512, n=1024]):
  - bf16 matmul operands for qkv/scores/proj (FWL halves weight loads,
    halves SBUF/DMA traffic); AV runs f32r (probs from ACT exp).
  - attention software-pipelined one chunk ahead: PE order is
    s(0), s(1), av(0), s(2), av(1), ... so the scores feeding exp(c+1)
    are already in PSUM while ACT runs exp(c); scores pool bufs=2 +
    single av accumulator [65,2,1024] = exactly 8 PSUM banks.
  - one exp instruction per [128,1024] chunk (ACT activate overhead is
    ~352 cycles, so bigger tiles amortize it).
  - two warmth-keeper dummy matmuls per chunk keep the PE's HAM activity
    window busy during exp waits so the PE clock stays at 2.4 GHz
    (the v1 kernel sat at K=4/8 = 1.2 GHz for 140 us of its 223 us).
  - engine balance: ACT owns exp (the 8.4M-element softmax wall) + q/k
    psum eviction (hidden under qkv matmuls); DVE owns GroupNorm,
    vT eviction, attention-output eviction/normalize, fused proj
    eviction (psum + bias + residual in one scalar_tensor_tensor).
  - denominators via ones-column in vt (M=65); reciprocal on a [128,2,8]
    reshape (166 ns instead of 1.2 us); broadcast via DRAM round-trip.
  - GroupNorm: selection-matrix matmuls for group reduce, rsqrt via
    exp(-0.5*ln(var+eps)); per-channel affine folded into qkv weights.
"""
import sys

sys.path.insert(0, "/opt/trn_rl_repo")

import numpy as np

import concourse.bass as bass
import concourse.bacc as bacc
import concourse.tile as tile
from concourse import mybir
from concourse.bass_utils import run_bass_kernel_spmd

F32 = mybir.dt.float32
F32R = mybir.dt.float32r
BF16 = mybir.dt.bfloat16
AX = mybir.AxisListType
OP = mybir.AluOpType
AF = mybir.ActivationFunctionType

C = 512          # channels
N = 1024         # spatial positions (32*32)
HEADS = 8
HD = 64          # head dim
G = 32           # groups
GSZ = 16         # channels per group
EPS = 1e-6
NC4 = 4          # channel chunks of 128
NM8 = 8          # spatial chunks of 128
WQ0 = 0          # wpack col offsets
WK0 = 512
WV0 = 1024
WP0 = 1536
WPACK_COLS = 2048


def build_nc():
    nc = bacc.Bacc(None)
    x = nc.declare_dram_parameter("x", [C, N], F32, isOutput=False)
    wpack = nc.declare_dram_parameter("wpack", [C, WPACK_COLS], BF16, isOutput=False)
    gsel = nc.declare_dram_parameter("gsel", [C, G], F32, isOutput=False)
    gselT = nc.declare_dram_parameter("gselT", [G, C], F32, isOutput=False)
    bpack = nc.declare_dram_parameter("bpack", [128, 12], F32, isOutput=False)
    y = nc.declare_dram_parameter("y", [C, N], F32, isOutput=True)

    rdram = nc.dram_tensor("rdram", [HEADS, N], F32)

    with tile.TileContext(nc) as tc:
        with (
            tc.tile_pool(name="const", bufs=1) as const,
            tc.tile_pool(name="main", bufs=1) as main,
        ):
            # ---- Phase 0: input DMAs (x first - everything waits on it) ----
            x_sb = main.tile([128, NC4, N], F32)
            xv = x[:].rearrange("(c p) n -> p c n", p=128)
            for c4 in range(NC4):
                nc.sync.dma_start(x_sb[:, c4, :], xv[:, c4, :])
            gs_sb = const.tile([128, NC4, G], F32)
            nc.sync.dma_start(gs_sb[:], gsel[:].rearrange("(c p) g -> p c g", p=128))
            gt_sb = const.tile([G, C], F32)
            nc.sync.dma_start(gt_sb[:], gselT[:])
            bp_sb = const.tile([128, 12], F32)
            nc.sync.dma_start(bp_sb[:], bpack[:])
            wp_sb = const.tile([128, NC4, WPACK_COLS], BF16)
            wpv = wpack[:].rearrange("(c p) m -> p c m", p=128)

            # ---- Phase 1: GroupNorm stats -> xh (standardized x, bf16) ----
            cst = const.tile([128, NC4, 2], F32)       # per-channel sum | sumsq
            gsb = const.tile([G, 8], F32)              # group scratch
            eps_t = const.tile([G, 1], F32)
            nc.vector.memset(eps_t[:], EPS)
            gstats = const.tile([G, 2], F32)           # rs | -mean*rs
            chsc = const.tile([128, NC4, 2], F32)      # per-channel rs | bias
            xh_sb = main.tile([128, NC4, N], BF16)

            # prefire the Square table set first (the stats need it next);
            # the Ln/Exp set is loaded once after the squares finish
            nc.scalar.activation(gsb[:, 4:5], eps_t[:], AF.Square)

            with (
                tc.tile_pool(name="sq", bufs=2) as sqp,
                tc.tile_pool(name="pst", bufs=2, space="PSUM") as pst,
            ):
                for c4 in range(NC4):
                    nc.vector.tensor_reduce(
                        cst[:, c4, 0:1], x_sb[:, c4, :], axis=AX.X, op=OP.add
                    )
                    sq = sqp.tile([128, N], F32)
                    nc.scalar.activation(
                        sq[:], x_sb[:, c4, :], AF.Square,
                        accum_out=cst[:, c4, 1:2],
                    )
                # weights DMA dispatched from ACT after the stat squares so
                # the 2MB wpack transfer doesn't steal bandwidth from x
                for c4 in range(NC4):
                    nc.scalar.dma_start(wp_sb[:, c4, :], wpv[:, c4, :])
                gs_ps = pst.tile([G, 2], F32)
                for c4 in range(NC4):
                    nc.tensor.matmul(
                        gs_ps[:],
                        gs_sb[:, c4, :],
                        cst[:, c4, :],
                        start=(c4 == 0),
                        stop=(c4 == NC4 - 1),
                    )
                nc.vector.tensor_copy(gsb[:, 0:2], gs_ps[:])
                # mean = gsb[:,0], ex2 = gsb[:,1] (both already / 16384)
                nc.vector.tensor_mul(gsb[:, 2:3], gsb[:, 0:1], gsb[:, 0:1])
                nc.vector.tensor_sub(gsb[:, 3:4], gsb[:, 1:2], gsb[:, 2:3])
                nc.scalar.activation(gsb[:, 4:5], gsb[:, 3:4], AF.Ln, bias=eps_t[:])
                nc.scalar.activation(gstats[:, 0:1], gsb[:, 4:5], AF.Exp, scale=-0.5)
                nc.vector.tensor_mul(gsb[:, 6:7], gsb[:, 0:1], gstats[:, 0:1])
                nc.vector.tensor_scalar_mul(gstats[:, 1:2], gsb[:, 6:7], -1.0)
                for c4 in range(NC4):
                    cs_ps = pst.tile([128, 2], F32)
                    nc.tensor.matmul(
                        cs_ps[:],
                        gt_sb[:, c4 * 128:(c4 + 1) * 128],
                        gstats[:],
                        start=True,
                        stop=True,
                    )
                    nc.vector.tensor_copy(chsc[:, c4, :], cs_ps[:])
                for c4 in range(NC4):
                    nc.vector.tensor_scalar(
                        xh_sb[:, c4, :], x_sb[:, c4, :],
                        chsc[:, c4, 0:1], chsc[:, c4, 1:2],
                        op0=OP.mult, op1=OP.add,
                    )

            # ---- Phase 2: qkv ----
            q_sb = main.tile([128, NC4, N], BF16)
            k_sb = main.tile([128, NC4, N], BF16)
            vt_sb = main.tile([128, NM8, HEADS, HD + 1], F32R)
            ones64 = const.tile([128, NM8 * HEADS], F32)
            nc.vector.memset(ones64[:], 1.0)
            nc.vector.tensor_copy(
                vt_sb[:, :, :, HD],
                ones64[:].rearrange("p (a b) -> p a b", a=NM8),
            )

            # probs pool lives across phase 2 + 3: pair 0's first 14 exp
            # chunks run DURING the qkv phase (ACT is otherwise idle there
            # while attention is ACT-bound), so their pt tiles must survive
            # until the attention scope consumes them.
            probs_pool = tc.tile_pool(name="probs", bufs=18)
            probs = probs_pool.__enter__()
            ptE = {}
            rows = (slice(0, HD), slice(HD, 128))

            with (
                tc.tile_pool(name="pmm", bufs=1, space="PSUM") as pmm,
                tc.tile_pool(name="pvv", bufs=2, space="PSUM") as pvv,
                tc.tile_pool(name="pesc", bufs=2, space="PSUM") as pesc,
            ):
                # q/k m=0 first: pair 0's scores only need these
                for dst, woff, boff in ((q_sb, WQ0, 0), (k_sb, WK0, 4)):
                    ps = pmm.tile([128, N], F32, tag="qk")
                    for kc in range(NC4):
                        for nh in range(2):
                            nc.tensor.matmul(
                                ps[:, nh * 512:(nh + 1) * 512],
                                wp_sb[:, kc, woff:woff + 128],
                                xh_sb[:, kc, nh * 512:(nh + 1) * 512],
                                start=(kc == 0),
                                stop=(kc == NC4 - 1),
                            )
                    nc.scalar.activation(
                        dst[:, 0, :], ps[:], AF.Identity,
                        bias=bp_sb[:, boff:boff + 1],
                    )
                # vT: stationary xh spatial chunk, moving all v weights
                for mt in range(NM8):
                    ps = pvv.tile([128, 512], F32, tag="v")
                    for kc in range(NC4):
                        nc.tensor.matmul(
                            ps[:],
                            xh_sb[:, kc, mt * 128:(mt + 1) * 128],
                            wp_sb[:, kc, WV0:WV0 + C],
                            start=(kc == 0),
                            stop=(kc == NC4 - 1),
                        )
                    nc.vector.tensor_copy(
                        vt_sb[:, mt, :, 0:HD],
                        ps[:].rearrange("p (h c) -> p h c", h=HEADS),
                    )

                # q/k m=1..3 emitted one matmul at a time between the early
                # score chunks below, so the PE stays dense while ACT exps
                fill = []

                def qk_unit(dst, woff, boff, m):
                    ps = pmm.tile([128, N], F32, name=f"qk{woff}_{m}", tag="qk")
                    for kc in range(NC4):
                        for nh in range(2):
                            fill.append(lambda ps=ps, kc=kc, nh=nh, woff=woff, m=m: nc.tensor.matmul(
                                ps[:, nh * 512:(nh + 1) * 512],
                                wp_sb[:, kc, woff + m * 128:woff + (m + 1) * 128],
                                xh_sb[:, kc, nh * 512:(nh + 1) * 512],
                                start=(kc == 0),
                                stop=(kc == NC4 - 1),
                            ))
                    fill.append(lambda ps=ps, dst=dst, m=m, boff=boff: nc.scalar.activation(
                        dst[:, m, :], ps[:], AF.Identity,
                        bias=bp_sb[:, boff + m:boff + m + 1],
                    ))

                for m in range(1, 4):
                    qk_unit(q_sb, WQ0, 0, m)
                    qk_unit(k_sb, WK0, 4, m)

                # early pair-0 scores+exp (kq 0..6, both hh), interleaved
                # with the remaining qkv matmuls
                for c in range(14):
                    kq, hh = c // 2, c % 2
                    esc = pesc.tile([128, N], F32, name=f"esc{c}", tag="esc")
                    for nh in range(2):
                        nc.tensor.matmul(
                            esc[:, nh * 512:(nh + 1) * 512],
                            k_sb[rows[hh], 0, kq * 128:(kq + 1) * 128],
                            q_sb[rows[hh], 0, nh * 512:(nh + 1) * 512],
                            start=True,
                            stop=True,
                        )
                    pt = probs.tile([128, N], F32R, name=f"ptE{c}", tag="pt")
                    nc.scalar.activation(pt[:], esc[:], AF.Exp)
                    ptE[(kq, hh)] = pt
                    for _ in range(4):
                        if fill:
                            fill.pop(0)()
                while fill:
                    fill.pop(0)()

            # ---- Phase 3: attention ----
            aun_ch = main.tile([128, NC4, N], F32)      # unnormalized A
            a_sb = main.tile([128, NC4, N], BF16)       # normalized A

            with (
                tc.tile_pool(name="spool", bufs=2) as spool,
                tc.tile_pool(name="rpool", bufs=2) as rpool,
                tc.tile_pool(name="dpool", bufs=2) as dpool,
                tc.tile_pool(name="psc", bufs=2, space="PSUM") as psc,
                tc.tile_pool(name="pav", bufs=1, space="PSUM") as pav,
            ):
                pending = []   # emitted-later closures: av MMs run one chunk
                               # behind scores so exp(c+1) overlaps av(c)

                def flush():
                    while pending:
                        pending.pop(0)()

                for j in range(4):
                    av = pav.tile([HD + 1, 2, N], F32, name=f"av{j}", tag="av")
                    for kq in range(NM8):
                        for hh in range(2):
                            pre = ptE.get((kq, hh)) if j == 0 else None
                            if pre is not None:
                                pt = pre
                            else:
                                sps = psc.tile(
                                    [128, N], F32, name=f"sps{j}_{kq}_{hh}",
                                    tag="sps",
                                )
                                # small warmth keeper (~113ns) every other
                                # chunk; with them on every chunk the PE was
                                # the attention pacer (1.23us vs exp 1.11us)
                                if (kq + hh) % 2 == 0:
                                    nc.tensor.matmul(
                                        sps[:, 0:128],
                                        wp_sb[:, 0, 0:128],
                                        xh_sb[:, 0, 0:128],
                                        start=True,
                                        stop=True,
                                    )
                                for nh in range(2):
                                    nc.tensor.matmul(
                                        sps[:, nh * 512:(nh + 1) * 512],
                                        k_sb[rows[hh], j, kq * 128:(kq + 1) * 128],
                                        q_sb[rows[hh], j, nh * 512:(nh + 1) * 512],
                                        start=True,
                                        stop=True,
                                    )
                                pt = probs.tile(
                                    [128, N], F32R,
                                    name=f"pt{j}_{kq}_{hh}", tag="pt",
                                )
                                nc.scalar.activation(pt[:], sps[:], AF.Exp)

                            def mk_av(j=j, kq=kq, hh=hh, pt=pt, av=av):
                                def go():
                                    for nh in range(2):
                                        nc.tensor.matmul(
                                            av[:, hh, nh * 512:(nh + 1) * 512],
                                            vt_sb[:, kq, 2 * j + hh, :],
                                            pt[:, nh * 512:(nh + 1) * 512],
                                            start=(kq == 0),
                                            stop=(kq == NM8 - 1),
                                        )
                                return go
                            pending.append(mk_av())
                            if len(pending) > 1:
                                pending.pop(0)()
                    flush()

                    # pair end: evict av rows + d rows (psum can't be DMA'd,
                    # so stage each hh through SBUF, then sbuf->sbuf DMAs)
                    dd = dpool.tile([128, 2, 8], F32, name=f"dd{j}", tag="dd")
                    for hh in range(2):
                        s_t = spool.tile(
                            [HD + 1, N], F32, name=f"st{j}_{hh}", tag=f"st{hh}"
                        )
                        if j == 3 and hh == 0:
                            # last pair: ACT is done with exps, so run the
                            # two eviction copies on ACT+DVE in parallel
                            # (shortens the chain gating proj's kc=3)
                            nc.scalar.activation(s_t[:], av[:, hh, :], AF.Copy)
                        else:
                            nc.vector.tensor_copy(s_t[:], av[:, hh, :])
                        nc.sync.dma_start(dd[:, hh, :], s_t[HD:HD + 1, :])
                        nc.sync.dma_start(aun_ch[rows[hh], j, :], s_t[0:HD, :])
                    # reciprocal of the pair's denominators, then broadcast
                    rr = dpool.tile([128, 2, 8], F32, name=f"rr{j}", tag="rr")
                    nc.vector.reciprocal(rr[:], dd[:])
                    nc.sync.dma_start(
                        rdram[2 * j:2 * j + 2, :].rearrange("h (p f) -> p h f", f=8),
                        rr[:],
                    )
                    rt = rpool.tile([128, N], F32, name=f"rt{j}", tag="rt")
                    rsrc = rdram[2 * j:2 * j + 2, :]
                    nc.sync.dma_start(
                        rt[:],
                        bass.AP(
                            tensor=rsrc.tensor,
                            offset=rsrc.offset,
                            ap=[[N, 2], [0, HD], [1, N]],
                        ),
                    )
                    nc.vector.tensor_mul(a_sb[:, j, :], aun_ch[:, j, :], rt[:])

            probs_pool.__exit__(None, None, None)

            # ---- Phase 4: proj + bias(+v-bias fold) + residual ----
            with (
                tc.tile_pool(name="ppj", bufs=1, space="PSUM") as ppj,
                tc.tile_pool(name="ypool", bufs=2) as ypool,
            ):
                yv = y[:].rearrange("(m p) n -> m p n", p=128)
                # kc-outer with all 4 m-psums live: pairs 0-2 accumulate
                # while pair 3's normalize is still finishing on DVE/DMA,
                # and the PE never idles into a HAM re-throttle before proj
                pss = [
                    ppj.tile([128, N], F32, name=f"pj{m}", tag=f"pj{m}")
                    for m in range(4)
                ]
                for kc in range(NC4):
                    for m in range(4):
                        for nh in range(2):
                            nc.tensor.matmul(
                                pss[m][:, nh * 512:(nh + 1) * 512],
                                wp_sb[:, kc, WP0 + m * 128:WP0 + (m + 1) * 128],
                                a_sb[:, kc, nh * 512:(nh + 1) * 512],
                                start=(kc == 0),
                                stop=(kc == NC4 - 1),
                            )
                for m in range(4):
                    yt = ypool.tile([128, N], F32, tag="yt")
                    # fused eviction: (psum + bias) + residual in one DVE op
                    nc.vector.scalar_tensor_tensor(
                        yt[:], pss[m][:], bp_sb[:, 8 + m:9 + m], x_sb[:, m, :],
                        op0=OP.add, op1=OP.add,
                    )
                    nc.sync.dma_start(yv[m, :, :], yt[:])

    nc.compile()
    return nc


_NC_CACHE = None


def _get_nc():
    global _NC_CACHE
    if _NC_CACHE is None:
        _NC_CACHE = build_nc()
    return _NC_CACHE


def _to_bf16(a):
    import ml_dtypes
    return np.ascontiguousarray(a, np.float32).astype(ml_dtypes.bfloat16)


def _prep_host(norm_w, norm_b, qkv_w, qkv_b, proj_w, proj_b):
    g = norm_w.astype(np.float32)
    b = norm_b.astype(np.float32)
    Wq, Wk, Wv = qkv_w[0:C], qkv_w[C:2 * C], qkv_w[2 * C:3 * C]
    bq, bk, bv = qkv_b[0:C], qkv_b[C:2 * C], qkv_b[2 * C:3 * C]
    scale = np.float32(1.0 / np.sqrt(HD))

    WqT = (scale * (Wq * g[None, :])).T
    WkT = (Wk * g[None, :]).T
    WvT = (Wv * g[None, :]).T
    bq_eff = scale * (Wq @ b + bq)
    bk_eff = Wk @ b + bk
    pb_eff = proj_w @ (Wv @ b + bv) + proj_b

    cidx = np.arange(C)
    gsel = np.zeros((C, G), np.float32)
    gsel[cidx, cidx // GSZ] = np.float32(1.0 / (GSZ * N))
    gselT = np.zeros((G, C), np.float32)
    gselT[cidx // GSZ, cidx] = 1.0

    wpack = np.concatenate([WqT, WkT, WvT, proj_w.T], axis=1).astype(np.float32)
    assert wpack.shape == (C, WPACK_COLS)
    wpack_bf16 = _to_bf16(wpack)

    bpack = np.stack(
        [bq_eff.reshape(4, 128), bk_eff.reshape(4, 128),
         pb_eff.reshape(4, 128)], axis=0,
    ).reshape(12, 128).T.astype(np.float32)
    return (np.ascontiguousarray(wpack_bf16), np.ascontiguousarray(gsel), gselT,
            np.ascontiguousarray(bpack))


def make_in_maps(x, norm_w, norm_b, qkv_w, qkv_b, proj_w, proj_b):
    b_sz = x.shape[0]
    wpack, gsel, gselT, bpack = _prep_host(
        norm_w, norm_b, qkv_w, qkv_b, proj_w, proj_b
    )
    xf = np.ascontiguousarray(x.reshape(b_sz, C, N).astype(np.float32))
    return [
        {"x": xf[i], "wpack": wpack, "gsel": gsel, "gselT": gselT,
         "bpack": bpack}
        for i in range(b_sz)
    ]


def kernel(x, norm_w, norm_b, qkv_w, qkv_b, proj_w, proj_b):
    x, norm_w, norm_b, qkv_w, qkv_b, proj_w, proj_b = (
        np.asarray(a, dtype=np.float32)
        for a in (x, norm_w, norm_b, qkv_w, qkv_b, proj_w, proj_b)
    )
    b_sz, c, h, w = x.shape
    assert (b_sz, c, h * w) == (8, C, N)
    nc = _get_nc()
    in_maps = make_in_maps(x, norm_w, norm_b, qkv_w, qkv_b, proj_w, proj_b)
    res = run_bass_kernel_spmd(nc, in_maps, core_ids=list(range(b_sz)))
    out = np.stack([r["y"] for r in res.results], axis=0)
    return out.reshape(b_sz, C, h, w)



# revision 6
# speedup vs baseline: 1.0352x; 1.0352x over previous
"""Trainium2 Bass kernel for nn_AttentionBlock (GroupNorm + MHA + proj + residual).

Problem: x[8, 512, 32, 32] fp32; GroupNorm(32 groups) -> qkv (1x1 conv) ->
8-head attention over 1024 spatial positions -> proj -> residual.

Sharding: data-parallel over batch. 8 batch elements -> 8 NeuronCores,
one SPMD NEFF. No collectives.

v3 design (per-core, x as [c=512, n=1024]):
  - ACT is the single pacer: 64 exp chunks of [128,1024] (~67 us). All
    other ACT work (q/k psum evictions) moved to DVE so ACT runs pure
    exp back-to-back; all 16 j=0 exps start during the qkv phase.
  - AV runs fp8e4 DoubleRow: probs are written by ACT exp directly as
    fp8e4 (exp(s-6); the -6 shift cancels exactly in the softmax
    normalization and keeps exp outputs inside fp8e4's 448 max), and
    vT is evicted as fp8e4. DR contracts 2 kq-chunks (256 rows) per
    matmul, halving AV accumulation steps vs f32r.
  - q/k/scores stay bf16 (scores are PE-output-bound; fp8 wouldn't
    speed them and costs accuracy). qkv/proj matmuls stay bf16.
  - attention software-pipelined: scores(c+1) overlap exp(c); av DR
    matmuls run one kq-pair behind the exps that produce them.
  - warmth-keeper dummy matmuls keep the PE HAM activity window busy
    so the PE clock stays at 2.4 GHz during exp-paced stretches.
  - denominators via ones-column in vt (M=65); reciprocal on a
    [128,2,8] reshape; broadcast via DRAM round-trip.
  - GroupNorm: selection-matrix matmuls for group reduce, rsqrt via
    exp(-0.5*ln(var+eps)); per-channel affine folded into qkv weights.
"""
import sys

sys.path.insert(0, "/opt/trn_rl_repo")

import numpy as np

import concourse.bass as bass
import concourse.bacc as bacc
import concourse.tile as tile
from concourse import mybir
from concourse.bass_utils import run_bass_kernel_spmd

F32 = mybir.dt.float32
F32R = mybir.dt.float32r
BF16 = mybir.dt.bfloat16
F8 = mybir.dt.float8e4
AX = mybir.AxisListType
OP = mybir.AluOpType
AF = mybir.ActivationFunctionType
DR = mybir.MatmulPerfMode.DoubleRow

C = 512          # channels
N = 1024         # spatial positions (32*32)
HEADS = 8
HD = 64          # head dim
G = 32           # groups
GSZ = 16         # channels per group
EPS = 1e-6
NC4 = 4          # channel chunks of 128
NM8 = 8          # spatial chunks of 128
VTW = 66         # vt row width: 64 hd + ones + zero pad (16B stride align)
EXP_SHIFT = -6.0  # exp(s-6): cancels in softmax, keeps exp in fp8e4 range
WQ0 = 0          # wpack col offsets
WK0 = 512
WV0 = 1024
WP0 = 1536
WPACK_COLS = 2048


def build_nc():
    nc = bacc.Bacc(None)
    x = nc.declare_dram_parameter("x", [C, N], F32, isOutput=False)
    wpack = nc.declare_dram_parameter("wpack", [C, WPACK_COLS], BF16, isOutput=False)
    gsel = nc.declare_dram_parameter("gsel", [C, G], F32, isOutput=False)
    gselT = nc.declare_dram_parameter("gselT", [G, C], F32, isOutput=False)
    bpack = nc.declare_dram_parameter("bpack", [128, 12], F32, isOutput=False)
    y = nc.declare_dram_parameter("y", [C, N], F32, isOutput=True)

    rdram = nc.dram_tensor("rdram", [HEADS, N], F32)

    with tile.TileContext(nc) as tc:
        with (
            tc.tile_pool(name="const", bufs=1) as const,
            tc.tile_pool(name="main", bufs=1) as main,
        ):
            # ---- Phase 0: input DMAs (x first - everything waits on it) ----
            x_sb = main.tile([128, NC4, N], F32)
            xv = x[:].rearrange("(c p) n -> p c n", p=128)
            for c4 in range(NC4):
                nc.sync.dma_start(x_sb[:, c4, :], xv[:, c4, :])
            gs_sb = const.tile([128, NC4, G], F32)
            nc.sync.dma_start(gs_sb[:], gsel[:].rearrange("(c p) g -> p c g", p=128))
            gt_sb = const.tile([G, C], F32)
            nc.sync.dma_start(gt_sb[:], gselT[:])
            bp_sb = const.tile([128, 12], F32)
            nc.sync.dma_start(bp_sb[:], bpack[:])
            wp_sb = const.tile([128, NC4, WPACK_COLS], BF16)
            wpv = wpack[:].rearrange("(c p) m -> p c m", p=128)

            # ---- Phase 1: GroupNorm stats -> xh (standardized x, bf16) ----
            cst = const.tile([128, NC4, 2], F32)       # per-channel sum | sumsq
            gsb = const.tile([G, 8], F32)              # group scratch
            eps_t = const.tile([G, 1], F32)
            nc.vector.memset(eps_t[:], EPS)
            shift_t = const.tile([128, 1], F32)        # exp bias (-6)
            nc.vector.memset(shift_t[:], EXP_SHIFT)
            gstats = const.tile([G, 2], F32)           # rs | -mean*rs
            chsc = const.tile([128, NC4, 2], F32)      # per-channel rs | bias
            xh_sb = main.tile([128, NC4, N], BF16)

            # prefire the Square table set first (the stats need it next);
            # the Ln/Exp set is loaded once after the squares finish
            nc.scalar.activation(gsb[:, 4:5], eps_t[:], AF.Square)

            with (
                tc.tile_pool(name="sq", bufs=2) as sqp,
                tc.tile_pool(name="pst", bufs=2, space="PSUM") as pst,
            ):
                for c4 in range(NC4):
                    nc.vector.tensor_reduce(
                        cst[:, c4, 0:1], x_sb[:, c4, :], axis=AX.X, op=OP.add
                    )
                    sq = sqp.tile([128, N], F32)
                    nc.scalar.activation(
                        sq[:], x_sb[:, c4, :], AF.Square,
                        accum_out=cst[:, c4, 1:2],
                    )
                # weights DMA dispatched from ACT after the stat squares so
                # the 2MB wpack transfer doesn't steal bandwidth from x
                for c4 in range(NC4):
                    nc.scalar.dma_start(wp_sb[:, c4, :], wpv[:, c4, :])
                gs_ps = pst.tile([G, 2], F32)
                for c4 in range(NC4):
                    nc.tensor.matmul(
                        gs_ps[:],
                        gs_sb[:, c4, :],
                        cst[:, c4, :],
                        start=(c4 == 0),
                        stop=(c4 == NC4 - 1),
                    )
                nc.vector.tensor_copy(gsb[:, 0:2], gs_ps[:])
                # mean = gsb[:,0], ex2 = gsb[:,1] (both already / 16384)
                nc.vector.tensor_mul(gsb[:, 2:3], gsb[:, 0:1], gsb[:, 0:1])
                nc.vector.tensor_sub(gsb[:, 3:4], gsb[:, 1:2], gsb[:, 2:3])
                nc.scalar.activation(gsb[:, 4:5], gsb[:, 3:4], AF.Ln, bias=eps_t[:])
                nc.scalar.activation(gstats[:, 0:1], gsb[:, 4:5], AF.Exp, scale=-0.5)
                nc.vector.tensor_mul(gsb[:, 6:7], gsb[:, 0:1], gstats[:, 0:1])
                nc.vector.tensor_scalar_mul(gstats[:, 1:2], gsb[:, 6:7], -1.0)
                for c4 in range(NC4):
                    cs_ps = pst.tile([128, 2], F32)
                    nc.tensor.matmul(
                        cs_ps[:],
                        gt_sb[:, c4 * 128:(c4 + 1) * 128],
                        gstats[:],
                        start=True,
                        stop=True,
                    )
                    nc.vector.tensor_copy(chsc[:, c4, :], cs_ps[:])
                for c4 in range(NC4):
                    nc.vector.tensor_scalar(
                        xh_sb[:, c4, :], x_sb[:, c4, :],
                        chsc[:, c4, 0:1], chsc[:, c4, 1:2],
                        op0=OP.mult, op1=OP.add,
                    )

            # ---- Phase 2: qkv ----
            q_sb = main.tile([128, NC4, N], BF16)
            k_sb = main.tile([128, NC4, N], BF16)
            vt_sb = main.tile([128, NM8, HEADS, VTW], F8)
            nc.vector.memset(vt_sb[:, :, :, HD:HD + 1], 1.0)
            nc.vector.memset(vt_sb[:, :, :, HD + 1:VTW], 0.0)

            # probs pool lives across phase 2 + 3: all 16 of j=0's exp
            # chunks run DURING the qkv phase (ACT is otherwise idle there
            # while attention is ACT-bound), so their pt tiles must survive
            # until the attention scope consumes them. Tiles are kq-PAIRS
            # [128, 2, N] in fp8e4 to feed DoubleRow AV matmuls.
            probs_pool = tc.tile_pool(name="probs", bufs=12)
            probs = probs_pool.__enter__()
            ptE = {}
            rows = (slice(0, HD), slice(HD, 128))

            with (
                tc.tile_pool(name="pmm", bufs=1, space="PSUM") as pmm,
                tc.tile_pool(name="pvv", bufs=2, space="PSUM") as pvv,
                tc.tile_pool(name="pesc", bufs=2, space="PSUM") as pesc,
            ):
                # q/k m=0 first: pair 0's scores only need these
                for dst, woff, boff in ((q_sb, WQ0, 0), (k_sb, WK0, 4)):
                    ps = pmm.tile([128, N], F32, tag="qk")
                    for kc in range(NC4):
                        for nh in range(2):
                            nc.tensor.matmul(
                                ps[:, nh * 512:(nh + 1) * 512],
                                wp_sb[:, kc, woff:woff + 128],
                                xh_sb[:, kc, nh * 512:(nh + 1) * 512],
                                start=(kc == 0),
                                stop=(kc == NC4 - 1),
                            )
                    nc.vector.tensor_scalar_add(
                        dst[:, 0, :], ps[:], bp_sb[:, boff:boff + 1]
                    )
                # vT: stationary xh spatial chunk, moving all v weights
                for mt in range(NM8):
                    ps = pvv.tile([128, 512], F32, tag="v")
                    for kc in range(NC4):
                        nc.tensor.matmul(
                            ps[:],
                            xh_sb[:, kc, mt * 128:(mt + 1) * 128],
                            wp_sb[:, kc, WV0:WV0 + C],
                            start=(kc == 0),
                            stop=(kc == NC4 - 1),
                        )
                    nc.vector.tensor_copy(
                        vt_sb[:, mt, :, 0:HD],
                        ps[:].rearrange("p (h c) -> p h c", h=HEADS),
                    )

                # q/k m=1..3 emitted one matmul at a time between the early
                # score chunks below, so the PE stays dense while ACT exps
                fill = []

                def qk_unit(dst, woff, boff, m):
                    ps = pmm.tile([128, N], F32, name=f"qk{woff}_{m}", tag="qk")
                    for kc in range(NC4):
                        for nh in range(2):
                            fill.append(lambda ps=ps, kc=kc, nh=nh, woff=woff, m=m: nc.tensor.matmul(
                                ps[:, nh * 512:(nh + 1) * 512],
                                wp_sb[:, kc, woff + m * 128:woff + (m + 1) * 128],
                                xh_sb[:, kc, nh * 512:(nh + 1) * 512],
                                start=(kc == 0),
                                stop=(kc == NC4 - 1),
                            ))
                    fill.append(lambda ps=ps, dst=dst, m=m, boff=boff: nc.vector.tensor_scalar_add(
                        dst[:, m, :], ps[:], bp_sb[:, boff + m:boff + m + 1]
                    ))

                for m in range(1, 4):
                    qk_unit(q_sb, WQ0, 0, m)
                    qk_unit(k_sb, WK0, 4, m)

                # early pair-0 scores+exp (ALL of j=0: kq 0..7, both hh),
                # interleaved with the remaining qkv matmuls
                for c in range(16):
                    kq, hh = c // 2, c % 2
                    esc = pesc.tile([128, N], F32, name=f"esc{c}", tag="esc")
                    for nh in range(2):
                        nc.tensor.matmul(
                            esc[:, nh * 512:(nh + 1) * 512],
                            k_sb[rows[hh], 0, kq * 128:(kq + 1) * 128],
                            q_sb[rows[hh], 0, nh * 512:(nh + 1) * 512],
                            start=True,
                            stop=True,
                        )
                    key = (kq // 2, hh)
                    if kq % 2 == 0:
                        ptp = probs.tile([128, 2, N], F8, name=f"ptE{key}", tag="pt")
                        ptE[key] = ptp
                    else:
                        ptp = ptE[key]
                    nc.scalar.activation(
                        ptp[:, kq % 2, :], esc[:], AF.Exp, bias=shift_t[:]
                    )
                    for _ in range(4):
                        if fill:
                            fill.pop(0)()
                while fill:
                    fill.pop(0)()

            # ---- Phase 3: attention ----
            aun_ch = main.tile([128, NC4, N], F32)      # unnormalized A
            a_sb = main.tile([128, NC4, N], BF16)       # normalized A

            with (
                tc.tile_pool(name="spool", bufs=2) as spool,
                tc.tile_pool(name="rpool", bufs=2) as rpool,
                tc.tile_pool(name="dpool", bufs=2) as dpool,
                tc.tile_pool(name="psc", bufs=2, space="PSUM") as psc,
                tc.tile_pool(name="pav", bufs=1, space="PSUM") as pav,
            ):
                pending = []   # emitted-later closures: av DR matmuls run
                               # one kq-pair behind the exps producing them

                def flush():
                    while pending:
                        pending.pop(0)()

                for j in range(4):
                    av = pav.tile([VTW, 2, N], F32, name=f"av{j}", tag="av")
                    for kq in range(NM8):
                        for hh in range(2):
                            key = (kq // 2, hh)
                            if j == 0:
                                ptp = ptE[key]
                            else:
                                sps = psc.tile(
                                    [128, N], F32, name=f"sps{j}_{kq}_{hh}",
                                    tag="sps",
                                )
                                # small warmth keeper (~113ns) every other
                                # chunk keeps the PE HAM window busy
                                if (kq + hh) % 2 == 0:
                                    nc.tensor.matmul(
                                        sps[:, 0:128],
                                        wp_sb[:, 0, 0:128],
                                        xh_sb[:, 0, 0:128],
                                        start=True,
                                        stop=True,
                                    )
                                for nh in range(2):
                                    nc.tensor.matmul(
                                        sps[:, nh * 512:(nh + 1) * 512],
                                        k_sb[rows[hh], j, kq * 128:(kq + 1) * 128],
                                        q_sb[rows[hh], j, nh * 512:(nh + 1) * 512],
                                        start=True,
                                        stop=True,
                                    )
                                if kq % 2 == 0:
                                    ptp = probs.tile(
                                        [128, 2, N], F8,
                                        name=f"pt{j}_{kq // 2}_{hh}", tag="pt",
                                    )
                                    ptE[key] = ptp
                                else:
                                    ptp = ptE[key]
                                nc.scalar.activation(
                                    ptp[:, kq % 2, :], sps[:], AF.Exp,
                                    bias=shift_t[:],
                                )

                            if kq % 2 == 1:
                                def mk_av(j=j, p=kq // 2, hh=hh, ptp=ptp, av=av):
                                    def go():
                                        # psum accumulation groups are 2KB-
                                        # bank granular: start only on the
                                        # first 256-col chunk of each bank,
                                        # stop on the last
                                        for nh in range(4):
                                            nc.tensor.matmul(
                                                av[:, hh, nh * 256:(nh + 1) * 256],
                                                vt_sb[:, 2 * p:2 * p + 2, 2 * j + hh, :],  # [128,2,66]
                                                ptp[:, :, nh * 256:(nh + 1) * 256],
                                                start=(p == 0 and nh % 2 == 0),
                                                stop=(p == 3 and nh % 2 == 1),
                                                perf_mode=DR,
                                            )
                                    return go
                                pending.append(mk_av())
                                if len(pending) > 1:
                                    pending.pop(0)()
                    flush()

                    # pair end: evict av rows + d rows (psum can't be DMA'd,
                    # so stage each hh through SBUF, then sbuf->sbuf DMAs)
                    dd = dpool.tile([128, 2, 8], F32, name=f"dd{j}", tag="dd")
                    for hh in range(2):
                        s_t = spool.tile(
                            [HD + 1, N], F32, name=f"st{j}_{hh}", tag=f"st{hh}"
                        )
                        if j == 3 and hh == 0:
                            # last pair: ACT is done with exps, so run the
                            # two eviction copies on ACT+DVE in parallel
                            # (shortens the chain gating proj's kc=3)
                            nc.scalar.activation(s_t[:], av[0:HD + 1, hh, :], AF.Copy)
                        else:
                            nc.vector.tensor_copy(s_t[:], av[0:HD + 1, hh, :])
                        nc.sync.dma_start(dd[:, hh, :], s_t[HD:HD + 1, :])
                        nc.sync.dma_start(aun_ch[rows[hh], j, :], s_t[0:HD, :])
                    # reciprocal of the pair's denominators, then broadcast
                    rr = dpool.tile([128, 2, 8], F32, name=f"rr{j}", tag="rr")
                    nc.vector.reciprocal(rr[:], dd[:])
                    nc.sync.dma_start(
                        rdram[2 * j:2 * j + 2, :].rearrange("h (p f) -> p h f", f=8),
                        rr[:],
                    )
                    rt = rpool.tile([128, N], F32, name=f"rt{j}", tag="rt")
                    rsrc = rdram[2 * j:2 * j + 2, :]
                    nc.sync.dma_start(
                        rt[:],
                        bass.AP(
                            tensor=rsrc.tensor,
                            offset=rsrc.offset,
                            ap=[[N, 2], [0, HD], [1, N]],
                        ),
                    )
                    nc.vector.tensor_mul(a_sb[:, j, :], aun_ch[:, j, :], rt[:])

            probs_pool.__exit__(None, None, None)

            # ---- Phase 4: proj + bias(+v-bias fold) + residual ----
            with (
                tc.tile_pool(name="ppj", bufs=1, space="PSUM") as ppj,
                tc.tile_pool(name="ypool", bufs=2) as ypool,
            ):
                yv = y[:].rearrange("(m p) n -> m p n", p=128)
                # kc-outer with all 4 m-psums live: pairs 0-2 accumulate
                # while pair 3's normalize is still finishing on DVE/DMA,
                # and the PE never idles into a HAM re-throttle before proj
                pss = [
                    ppj.tile([128, N], F32, name=f"pj{m}", tag=f"pj{m}")
                    for m in range(4)
                ]
                for kc in range(NC4):
                    for m in range(4):
                        for nh in range(2):
                            nc.tensor.matmul(
                                pss[m][:, nh * 512:(nh + 1) * 512],
                                wp_sb[:, kc, WP0 + m * 128:WP0 + (m + 1) * 128],
                                a_sb[:, kc, nh * 512:(nh + 1) * 512],
                                start=(kc == 0),
                                stop=(kc == NC4 - 1),
                            )
                for m in range(4):
                    yt = ypool.tile([128, N], F32, tag="yt")
                    # fused eviction: (psum + bias) + residual in one DVE op
                    nc.vector.scalar_tensor_tensor(
                        yt[:], pss[m][:], bp_sb[:, 8 + m:9 + m], x_sb[:, m, :],
                        op0=OP.add, op1=OP.add,
                    )
                    nc.sync.dma_start(yv[m, :, :], yt[:])

    nc.compile()
    return nc


_NC_CACHE = None


def _get_nc():
    global _NC_CACHE
    if _NC_CACHE is None:
        _NC_CACHE = build_nc()
    return _NC_CACHE


def _to_bf16(a):
    import ml_dtypes
    return np.ascontiguousarray(a, np.float32).astype(ml_dtypes.bfloat16)


def _prep_host(norm_w, norm_b, qkv_w, qkv_b, proj_w, proj_b):
    g = norm_w.astype(np.float32)
    b = norm_b.astype(np.float32)
    Wq, Wk, Wv = qkv_w[0:C], qkv_w[C:2 * C], qkv_w[2 * C:3 * C]
    bq, bk, bv = qkv_b[0:C], qkv_b[C:2 * C], qkv_b[2 * C:3 * C]
    scale = np.float32(1.0 / np.sqrt(HD))

    WqT = (scale * (Wq * g[None, :])).T
    WkT = (Wk * g[None, :]).T
    WvT = (Wv * g[None, :]).T
    bq_eff = scale * (Wq @ b + bq)
    bk_eff = Wk @ b + bk
    pb_eff = proj_w @ (Wv @ b + bv) + proj_b

    cidx = np.arange(C)
    gsel = np.zeros((C, G), np.float32)
    gsel[cidx, cidx // GSZ] = np.float32(1.0 / (GSZ * N))
    gselT = np.zeros((G, C), np.float32)
    gselT[cidx // GSZ, cidx] = 1.0

    wpack = np.concatenate([WqT, WkT, WvT, proj_w.T], axis=1).astype(np.float32)
    assert wpack.shape == (C, WPACK_COLS)
    wpack_bf16 = _to_bf16(wpack)

    bpack = np.stack(
        [bq_eff.reshape(4, 128), bk_eff.reshape(4, 128),
         pb_eff.reshape(4, 128)], axis=0,
    ).reshape(12, 128).T.astype(np.float32)
    return (np.ascontiguousarray(wpack_bf16), np.ascontiguousarray(gsel), gselT,
            np.ascontiguousarray(bpack))


def make_in_maps(x, norm_w, norm_b, qkv_w, qkv_b, proj_w, proj_b):
    b_sz = x.shape[0]
    wpack, gsel, gselT, bpack = _prep_host(
        norm_w, norm_b, qkv_w, qkv_b, proj_w, proj_b
    )
    xf = np.ascontiguousarray(x.reshape(b_sz, C, N).astype(np.float32))
    return [
        {"x": xf[i], "wpack": wpack, "gsel": gsel, "gselT": gselT,
         "bpack": bpack}
        for i in range(b_sz)
    ]


def kernel(x, norm_w, norm_b, qkv_w, qkv_b, proj_w, proj_b):
    x, norm_w, norm_b, qkv_w, qkv_b, proj_w, proj_b = (
        np.asarray(a, dtype=np.float32)
        for a in (x, norm_w, norm_b, qkv_w, qkv_b, proj_w, proj_b)
    )
    b_sz, c, h, w = x.shape
    assert (b_sz, c, h * w) == (8, C, N)
    nc = _get_nc()
    in_maps = make_in_maps(x, norm_w, norm_b, qkv_w, qkv_b, proj_w, proj_b)
    res = run_bass_kernel_spmd(nc, in_maps, core_ids=list(range(b_sz)))
    out = np.stack([r["y"] for r in res.results], axis=0)
    return out.reshape(b_sz, C, h, w)
